# revision 16
# baseline (speedup 1.0000x reference)
"""EvidentialMLPEdge GNN kernel for 8 Trainium2 NeuronCores.

Design (per core, edges sharded 8 ways, node table replicated):
  Stage A: h = tanh(node_x @ proj_w + proj_b), computed from a host-transposed
           node_x^T via bf16 matmuls; h rows written to a DRAM table
           h_pad[n] = [h[n] (64 bf16) || zeros (64 bf16)]  (256B rows).
  Stage B: per 512-edge tile, two transposing dma_gathers (custom GPSIMD
           instruction) produce z_u^T / z_v^T feature-major [128, 512] tiles
           directly (real features on partitions 0-63, zero pad on 64-127).
           |z_u - z_v| and z_u*z_v are built with DVE/ACT into a DM tile;
           three accumulating matmuls with weight blocks [A;0],[B;0],[C;D]
           give hidden^T; ACT relu; per-128-edge matmuls with lhsT=hidden^T
           give edge-major logits; wide edge-major post-processing computes
           softplus/alpha/uncertainty/probs and DMAs them out.
  int16 gather indices can't reach 50000, so edges are bucketed by
  (u >= 25600, v >= 25600) into 4 groups on the host; hi-bucket gathers use a
  byte-offset view of the table. Outputs are written in slot order and
  unpermuted on the host.
"""

import numpy as np
import ml_dtypes

import concourse.bacc as bacc
import concourse.bass as bass
import concourse.mybir as mybir
from concourse.tile import TileContext
from concourse.bass_utils import run_bass_kernel_spmd
from concourse.masks import make_identity

N_NODES = 50000
N_EDGES = 800000
IN_DIM = 256
HID = 64
N_CORES = 8

TOK = 50048            # padded token count (multiple of 128)
SPLIT = 25600          # lo/hi table split (multiple of 128, both halves < 32768)
TILE = 512             # edges per gather tile
BATCH_TILES = 8        # tiles per post-processing batch
BATCH = TILE * BATCH_TILES  # 4096 edges

bf16 = mybir.dt.bfloat16
f32 = mybir.dt.float32
i16 = mybir.dt.int16

LAST_NC = None          # the built Bass program of the last kernel() call
LAST_EXEC_WALL_NS = None


def _split_excess_waits(nc, maxw=1):
    """Walrus only accepts 1 sync-wait per instruction here; hoist extras
    onto NOPs inserted just before."""
    counter = 0
    for _, bbobj in nc.bb_map.items():
        insts = bbobj.bb.instructions
        i = 0
        while i < len(insts):
            inst = insts[i]
            si = inst.sync_info
            if si is not None and si.on_wait and len(si.on_wait) > maxw:
                waits = list(si.on_wait)
                si.on_wait = waits[-maxw:]
                extra = waits[:-maxw]
                nops = []
                for j in range(0, len(extra), maxw):
                    counter += 1
                    nop = mybir.InstNoOp(name=f'I-ws-{counter}', ins=[], outs=[])
                    nop.engine = inst.engine
                    nop.sync_info = mybir.SyncInfo(on_wait=extra[j:j + maxw], on_update=[])
                    nops.append(nop)
                    nc.register_instruction(nop, overwrite=True)
                insts[i:i] = nops
                i += len(nops)
            i += 1
    return counter


def _build_nc(ntiles_per_bucket):
    """Build the SPMD program. ntiles_per_bucket: [n00, n01, n10, n11] tiles
    (identical on every core)."""
    ntiles = sum(ntiles_per_bucket)
    L = ntiles * TILE                      # padded edge slots per core
    nbatch = (ntiles + BATCH_TILES - 1) // BATCH_TILES
    assert ntiles % BATCH_TILES == 0

    nc = bacc.Bacc('TRN2', num_swdge_queues=4)

    xT = nc.declare_dram_parameter('xT', [IN_DIM, N_NODES], f32, isOutput=False)
    idxu = nc.declare_dram_parameter('idxu', [128, L // 16], i16, isOutput=False)
    idxv = nc.declare_dram_parameter('idxv', [128, L // 16], i16, isOutput=False)
    pw = nc.declare_dram_parameter('pw', [IN_DIM, HID], bf16, isOutput=False)
    pb = nc.declare_dram_parameter('pb', [HID, 1], f32, isOutput=False)
    w1u = nc.declare_dram_parameter('w1u', [128, HID], bf16, isOutput=False)
    w1v = nc.declare_dram_parameter('w1v', [128, HID], bf16, isOutput=False)
    w1dm = nc.declare_dram_parameter('w1dm', [128, HID], bf16, isOutput=False)
    b1 = nc.declare_dram_parameter('b1', [HID, 1], f32, isOutput=False)
    w2 = nc.declare_dram_parameter('w2', [HID, 2], bf16, isOutput=False)
    b2row = nc.declare_dram_parameter('b2row', [128, 64], f32, isOutput=False)

    ev_d = nc.declare_dram_parameter('ev', [L, 2], f32, isOutput=True)
    al_d = nc.declare_dram_parameter('al', [L, 2], f32, isOutput=True)
    un_d = nc.declare_dram_parameter('un', [L], f32, isOutput=True)
    pr_d = nc.declare_dram_parameter('pr', [L, 2], f32, isOutput=True)

    # bucket id per tile, in processing order
    tile_bucket = []
    for b, n in enumerate(ntiles_per_bucket):
        tile_bucket += [b] * n

    with TileContext(nc) as tc:
        with (
            tc.tile_pool(name='const', bufs=1) as cpool,
            tc.tile_pool(name='dram', bufs=1, space='DRAM') as dpool,
            tc.tile_pool(name='sa', bufs=3) as sa_pool,
            tc.tile_pool(name='ps', bufs=2, space='PSUM') as psum_pool,
            tc.tile_pool(name='sb', bufs=4) as sb_pool,
            tc.tile_pool(name='post', bufs=2) as post_pool,
        ):
            sa_psum = psum_pool
            sb_psum = psum_pool
            # ---- constants ----
            pw_a = cpool.tile([128, HID], bf16)
            nc.sync.dma_start(out=pw_a[:], in_=pw[0:128, :])
            pw_b = cpool.tile([128, HID], bf16)
            nc.sync.dma_start(out=pw_b[:], in_=pw[128:256, :])
            pb_sb = cpool.tile([HID, 1], f32)
            nc.sync.dma_start(out=pb_sb[:], in_=pb[:])
            w1u_sb = cpool.tile([128, HID], bf16)
            nc.sync.dma_start(out=w1u_sb[:], in_=w1u[:])
            w1v_sb = cpool.tile([128, HID], bf16)
            nc.sync.dma_start(out=w1v_sb[:], in_=w1v[:])
            w1dm_sb = cpool.tile([128, HID], bf16)
            nc.sync.dma_start(out=w1dm_sb[:], in_=w1dm[:])
            b1_sb = cpool.tile([HID, 1], f32)
            nc.sync.dma_start(out=b1_sb[:], in_=b1[:])
            w2_sb = cpool.tile([HID, 2], bf16)
            nc.sync.dma_start(out=w2_sb[:], in_=w2[:])
            b2_sb = cpool.tile([128, 64], f32)
            nc.sync.dma_start(out=b2_sb[:], in_=b2row[:])
            ident = cpool.tile([64, 64], bf16)
            make_identity(nc, ident[:])
            idxu_sb = cpool.tile([128, L // 16], i16)
            nc.sync.dma_start(out=idxu_sb[:], in_=idxu[:])
            idxv_sb = cpool.tile([128, L // 16], i16)
            nc.sync.dma_start(out=idxv_sb[:], in_=idxv[:])

            h_pad = dpool.tile([TOK, 128], bf16)   # DRAM node table

            # ---- stage A: h = tanh(x @ W + b), write padded rows ----
            CH = 512
            nch = (N_NODES + CH - 1) // CH          # 98 chunks (last partial)
            for ci in range(nch):
                n0 = ci * CH
                nn = min(CH, N_NODES - n0)
                nt = (nn + 127) // 128              # node sub-blocks of 128
                xa = sa_pool.tile([128, CH], bf16, tag='xa')
                xb = sa_pool.tile([128, CH], bf16, tag='xb')
                # SWDGE cast-DMA f32 -> bf16
                nc.gpsimd.dma_start(out=xa[:, :nn], in_=xT[0:128, n0:n0 + nn])
                nc.gpsimd.dma_start(out=xb[:, :nn], in_=xT[128:256, n0:n0 + nn])
                ps_hT = sa_psum.tile([HID, CH], f32, tag='hT', space='PSUM')
                nc.tensor.matmul(out=ps_hT[:, :nn], lhsT=pw_a[:],
                                 rhs=xa[:, :nn], start=True, stop=False)
                nc.tensor.matmul(out=ps_hT[:, :nn], lhsT=pw_b[:],
                                 rhs=xb[:, :nn], start=False, stop=True)
                hT_sb = sa_pool.tile([HID, CH], bf16, tag='hT_sb')
                nc.scalar.activation(out=hT_sb[:, :nn], in_=ps_hT[:, :nn],
                                     func=mybir.ActivationFunctionType.Tanh,
                                     bias=pb_sb[:, 0:1])
                # transpose to node-major and pad columns with zeros
                stage = sa_pool.tile([128, 4 * 128], bf16, tag='stage')
                nc.vector.memset(stage[:], 0.0)
                ps_T = sa_psum.tile([128, 256], bf16, tag='psT', space='PSUM')
                for t in range(nt):
                    c0 = t * 128
                    cw = min(128, nn - c0)
                    nc.tensor.transpose(out=ps_T[:cw, (t % 4) * 64:(t % 4) * 64 + 64],
                                        in_=hT_sb[:, c0:c0 + cw],
                                        identity=ident[:])
                    nc.vector.tensor_copy(out=stage[:cw, t * 128:t * 128 + 64],
                                          in_=ps_T[:cw, (t % 4) * 64:(t % 4) * 64 + 64])
                # one DMA: rows n0..n0+nn of h_pad
                h_view = h_pad[:]
                out_ap = h_view.rearrange("(c p) f -> p c f", p=128)  # [128, TOK/128, 128]
                nc.sync.dma_start(
                    out=out_ap[:, n0 // 128:n0 // 128 + nt, :],
                    in_=stage[:].rearrange("p (c f) -> p c f", f=128)[:, 0:nt, :],
                )

            # ---- stage B ----
            tbl_lo = h_pad[0:SPLIT, :]
            tbl_hi = h_pad[SPLIT:TOK, :]

            nbatches = ntiles // BATCH_TILES
            for bi in range(nbatches):
                stag = post_pool.tile([128, 2 * BATCH_TILES * 4], f32, tag='stag')
                for ti in range(BATCH_TILES):
                    t = bi * BATCH_TILES + ti
                    bucket = tile_bucket[t]
                    ub, vb = bucket >> 1, bucket & 1
                    s = t * TILE // 16
                    U = sb_pool.tile([128, TILE], bf16, tag='U')
                    V = sb_pool.tile([128, TILE], bf16, tag='V')
                    nc.gpsimd.dma_gather(
                        out_ap=U[:].rearrange("p (c n) -> p c n", c=1),
                        in_ap=(tbl_hi if ub else tbl_lo),
                        idxs_ap=idxu_sb[:, s:s + TILE // 16],
                        num_idxs=TILE, num_idxs_reg=TILE, elem_size=128,
                        transpose=True, queue_num=(2 * t) % 4,
                    )
                    nc.gpsimd.dma_gather(
                        out_ap=V[:].rearrange("p (c n) -> p c n", c=1),
                        in_ap=(tbl_hi if vb else tbl_lo),
                        idxs_ap=idxv_sb[:, s:s + TILE // 16],
                        num_idxs=TILE, num_idxs_reg=TILE, elem_size=128,
                        transpose=True, queue_num=(2 * t + 1) % 4,
                    )
                    DM = sb_pool.tile([128, TILE], bf16, tag='DM')
                    nc.vector.tensor_tensor(out=DM[0:HID, :], in0=U[0:HID, :],
                                            in1=V[0:HID, :],
                                            op=mybir.AluOpType.subtract)
                    nc.scalar.activation(out=DM[0:HID, :], in_=DM[0:HID, :],
                                         func=mybir.ActivationFunctionType.Abs)
                    nc.vector.tensor_tensor(out=DM[HID:128, :], in0=U[0:HID, :],
                                            in1=V[0:HID, :],
                                            op=mybir.AluOpType.mult)
                    ps_h = sb_psum.tile([HID, TILE], f32, tag='psh', space='PSUM')
                    nc.tensor.matmul(out=ps_h[:], lhsT=w1u_sb[:], rhs=U[:],
                                     start=True, stop=False)
                    nc.tensor.matmul(out=ps_h[:], lhsT=w1v_sb[:], rhs=V[:],
                                     start=False, stop=False)
                    nc.tensor.matmul(out=ps_h[:], lhsT=w1dm_sb[:], rhs=DM[:],
                                     start=False, stop=True)
                    hT = sb_pool.tile([HID, TILE], bf16, tag='hT2')
                    nc.scalar.activation(out=hT[:], in_=ps_h[:],
                                         func=mybir.ActivationFunctionType.Relu,
                                         bias=b1_sb[:, 0:1])
                    ps_lg = sb_psum.tile([128, 8], f32, tag='pslg', space='PSUM')
                    for b in range(4):
                        nc.tensor.matmul(out=ps_lg[:, 2 * b:2 * b + 2],
                                         lhsT=hT[:, b * 128:(b + 1) * 128],
                                         rhs=w2_sb[:], start=True, stop=True)
                    nc.vector.tensor_copy(out=stag[:, ti * 8:ti * 8 + 8], in_=ps_lg[:])

                # ---- post-processing on [128, 64] (4096 edges) ----
                W = BATCH_TILES * 4  # 32 column-pairs
                nc.vector.tensor_tensor(out=stag[:], in0=stag[:], in1=b2_sb[:],
                                        op=mybir.AluOpType.add)
                ex_sb = post_pool.tile([128, 2 * W], f32, tag='ex')
                nc.scalar.activation(out=ex_sb[:], in_=stag[:],
                                     func=mybir.ActivationFunctionType.Exp)
                ev_sb = post_pool.tile([128, 2 * W], f32, tag='ev')
                nc.scalar.activation(out=ev_sb[:], in_=ex_sb[:],
                                     func=mybir.ActivationFunctionType.Ln,
                                     bias=1.0)
                al_sb = post_pool.tile([128, 2 * W], f32, tag='al')
                nc.vector.tensor_scalar_add(out=al_sb[:], in0=ev_sb[:], scalar1=1.0)
                S_sb = post_pool.tile([128, W], f32, tag='S')
                al3 = al_sb[:].rearrange("p (j k) -> p j k", k=2)
                nc.vector.tensor_tensor(out=S_sb[:], in0=al3[:, :, 0],
                                        in1=al3[:, :, 1], op=mybir.AluOpType.add)
                R_sb = post_pool.tile([128, W], f32, tag='R')
                nc.vector.reciprocal(out=R_sb[:], in_=S_sb[:])
                un_sb = post_pool.tile([128, W], f32, tag='un')
                nc.vector.tensor_scalar_mul(out=un_sb[:], in0=R_sb[:], scalar1=2.0)
                pr_sb = post_pool.tile([128, 2 * W], f32, tag='pr')
                pr3 = pr_sb[:].rearrange("p (j k) -> p j k", k=2)
                nc.vector.tensor_tensor(out=pr3[:, :, 0], in0=al3[:, :, 0],
                                        in1=R_sb[:], op=mybir.AluOpType.mult)
                nc.vector.tensor_tensor(out=pr3[:, :, 1], in0=al3[:, :, 1],
                                        in1=R_sb[:], op=mybir.AluOpType.mult)
                # ---- output DMAs; slot s = bi*BATCH + p*W + j ----
                e0 = bi * BATCH
                ev_ap = ev_d[:].rearrange("(b p j) k -> b p j k", p=128, j=W)
                al_ap = al_d[:].rearrange("(b p j) k -> b p j k", p=128, j=W)
                pr_ap = pr_d[:].rearrange("(b p j) k -> b p j k", p=128, j=W)
                un_ap = un_d[:].rearrange("(b p j) -> b p j", p=128, j=W)
                nc.sync.dma_start(out=ev_ap[bi], in_=ev_sb[:].rearrange("p (j k) -> p j k", k=2))
                nc.sync.dma_start(out=al_ap[bi], in_=al_sb[:].rearrange("p (j k) -> p j k", k=2))
                nc.sync.dma_start(out=pr_ap[bi], in_=pr_sb[:].rearrange("p (j k) -> p j k", k=2))
                nc.sync.dma_start(out=un_ap[bi], in_=un_sb[:])

    _split_excess_waits(nc)
    nc.finalize()
    return nc


def _wrap_idx(flat):
    """[n] int16 -> [128, n/16] wrapped in 16 partitions, replicated 8x."""
    n = flat.shape[0]
    out = np.zeros((128, n // 16), np.int16)
    w = flat.reshape(n // 16, 16).T
    for g in range(8):
        out[g * 16:(g + 1) * 16, :] = w
    return out


def kernel(edge_index, node_x, proj_w, proj_b, w1, b1, w2, b2):
    edge_index = np.asarray(edge_index)
    node_x = np.asarray(node_x, dtype=np.float32)
    proj_w = np.asarray(proj_w, dtype=np.float32)
    proj_b = np.asarray(proj_b, dtype=np.float32)
    w1 = np.asarray(w1, dtype=np.float32)
    b1 = np.asarray(b1, dtype=np.float32)
    w2 = np.asarray(w2, dtype=np.float32)
    b2 = np.asarray(b2, dtype=np.float32)

    E = edge_index.shape[1]
    Ec = E // N_CORES
    u_all = edge_index[0].astype(np.int64)
    v_all = edge_index[1].astype(np.int64)

    # ---- per-core bucketing ----
    core_edges = []          # per core: list of 4 arrays of global edge ids
    counts = np.zeros((N_CORES, 4), np.int64)
    for c in range(N_CORES):
        lo, hi = c * Ec, (c + 1) * Ec
        ids = np.arange(lo, hi)
        ub = (u_all[lo:hi] >= SPLIT).astype(np.int64)
        vb = (v_all[lo:hi] >= SPLIT).astype(np.int64)
        bk = ub * 2 + vb
        groups = [ids[bk == b] for b in range(4)]
        core_edges.append(groups)
        counts[c] = [len(g) for g in groups]

    # identical tile counts across cores
    ntiles_b = [int(np.ceil(counts[:, b].max() / TILE)) for b in range(4)]
    ntot = sum(ntiles_b)
    ntot = int(np.ceil(ntot / BATCH_TILES)) * BATCH_TILES
    ntiles_b[0] += ntot - sum(ntiles_b)       # pad bucket 0 to round batches
    L = ntot * TILE

    nc = _build_nc(ntiles_b)

    xT = np.ascontiguousarray(node_x.T)
    pw_bf = proj_w.astype(ml_dtypes.bfloat16)
    A, B, C, D = w1[0:64], w1[64:128], w1[128:192], w1[192:256]
    zeros = np.zeros((64, HID), np.float32)
    w1u = np.concatenate([A, zeros]).astype(ml_dtypes.bfloat16)
    w1v = np.concatenate([B, zeros]).astype(ml_dtypes.bfloat16)
    w1dm = np.concatenate([C, D]).astype(ml_dtypes.bfloat16)
    w2_bf = w2.astype(ml_dtypes.bfloat16)
    b2row = np.broadcast_to(np.tile(b2, 32).reshape(1, 64), (128, 64)).astype(np.float32).copy()

    in_maps = []
    slot_edges = []
    W = BATCH_TILES * 4
    for c in range(N_CORES):
        # sequence of edge ids in tile order (pad with -1 -> dummy idx 0)
        seq = np.full(L, -1, np.int64)
        pos = 0
        for b in range(4):
            g = core_edges[c][b]
            seq[pos:pos + len(g)] = g
            pos += ntiles_b[b] * TILE
        # per-tile index arrays, bucket-local
        useq = np.where(seq >= 0, u_all[np.maximum(seq, 0)], 0)
        vseq = np.where(seq >= 0, v_all[np.maximum(seq, 0)], 0)
        # make dummies bucket-consistent: dummy edges use index 0 (lo)...
        # but tiles in hi buckets need a hi-range dummy -> use SPLIT
        tb = []
        for b, n in enumerate(ntiles_b):
            tb += [b] * n
        tb = np.array(tb)
        tile_of_slotseq = np.arange(L) // TILE
        ubk = (tb[tile_of_slotseq] >> 1).astype(bool)
        vbk = (tb[tile_of_slotseq] & 1).astype(bool)
        dummy = seq < 0
        useq = np.where(dummy & ubk, SPLIT, useq)
        vseq = np.where(dummy & vbk, SPLIT, vseq)
        u16 = np.where(ubk, useq - SPLIT, useq).astype(np.int16)
        v16 = np.where(vbk, vseq - SPLIT, vseq).astype(np.int16)

        in_maps.append({
            'xT': xT, 'idxu': _wrap_idx(u16), 'idxv': _wrap_idx(v16),
            'pw': pw_bf, 'pb': proj_b.reshape(HID, 1),
            'w1u': w1u, 'w1v': w1v, 'w1dm': w1dm,
            'b1': b1.reshape(HID, 1), 'w2': w2_bf, 'b2row': b2row,
        })
        # slot order: s = batch*BATCH + p*W + j ; tile t col i -> seq[t*TILE+i]
        # where j = (t % BATCH_TILES)*4 + i//128, p = i % 128
        t_idx = np.arange(L) // TILE
        i_idx = np.arange(L) % TILE
        s = (t_idx // BATCH_TILES) * BATCH + (i_idx % 128) * W \
            + (t_idx % BATCH_TILES) * 4 + (i_idx // 128)
        slot_edge = np.full(L, -1, np.int64)
        slot_edge[s] = seq
        slot_edges.append(slot_edge)

    import time as _time
    global LAST_NC, LAST_EXEC_WALL_NS
    LAST_NC = nc
    _t0 = _time.perf_counter()
    results = run_bass_kernel_spmd(nc, in_maps, list(range(N_CORES))).results
    LAST_EXEC_WALL_NS = (_time.perf_counter() - _t0) * 1e9

    evidence = np.empty((E, 2), np.float32)
    alpha = np.empty((E, 2), np.float32)
    uncertainty = np.empty((E,), np.float32)
    probs = np.empty((E, 2), np.float32)
    for c in range(N_CORES):
        se = slot_edges[c]
        m = se >= 0
        evidence[se[m]] = results[c]['ev'][m]
        alpha[se[m]] = results[c]['al'][m]
        uncertainty[se[m]] = results[c]['un'][m]
        probs[se[m]] = results[c]['pr'][m]
    return evidence, alpha, uncertainty, probs
